# revision 18
# baseline (speedup 1.0000x reference)
"""Multi-head attention (no qkv proj) + out_proj, sharded over 8 TRN2 cores.

Sharding: data-parallel over (batch, T-chunk): core i handles batch i//4,
query rows [ (i%4)*256, (i%4+1)*256 ).  Each core computes its full output
rows (all 16 heads + out_proj) independently -- no collectives needed.

Layout strategy ("T on the free dim" everywhere, zero on-device transposes):
  scoresT[s, t]  = K_h @ Q_h^T        lhsT = kT[64d, 128s]   rhs = qT[64d, 256t]
  rows[s, t]     = scoresT*scale + biasT(+mask)   (mask folded into biasT on
                   host; no max-subtract needed: |scores+bias| <= ~10)
  exp[s, t]      = exp(rows - 2)  in fp16 (prescale keeps exp < 65504; the
                   constant cancels in the softmax normalization)
  av[d(+1), t]   = [V_h | 1]^T @ exp  (65th row = softmax denominator)
  attnflatT[din, t] = av[0:64] * bcast(1/denom)   (reciprocal batched over
                   4 heads; broadcast across partitions via GPSIMD)
  outT[dout, t]  = W^T-chunks^T @ attnflatT + out_b  (bias per-partition,
                   fused into the PSUM->SBUF copy on the scalar engine)

Host pre-transposes q/k/w/bias so every DMA is contiguous.  Matmul operands
are fp16 (full-rate PE path + fast weight load); accumulation is fp32 in
PSUM; softmax bias add runs in fp32.
"""

import ml_dtypes
import numpy as np

import concourse.bass as bass
import concourse.mybir as mybir
import concourse.tile as tile
from concourse import bacc
from concourse.bass_utils import run_bass_kernel_spmd

F32 = mybir.dt.float32
import os as _os
F16 = mybir.dt.bfloat16 if _os.environ.get("MM_BF16") else mybir.dt.float16
BIAS_F16 = True  # attn_bias streamed as f32 (precision) vs f16 (half DMA)
BIAS_DT = F16 if BIAS_F16 else F32
NP16 = ml_dtypes.bfloat16 if _os.environ.get("MM_BF16") else np.float16
BIAS_NP = NP16 if BIAS_F16 else np.float32

P = 128          # partitions
T = 256          # query rows per core
S = 1024         # key length
H = 16           # heads
HD = 64          # head dim
DM = 1024        # d_model
NS = S // P      # 8 s-chunks
ND = DM // P     # 8 d_model-chunks
SCALE = HD ** -0.5
EXP_SHIFT = -2.0  # exp(x-2): keeps exp outputs < 65504 for fp16

AF = mybir.ActivationFunctionType
ALU = mybir.AluOpType


def build_bass():
    nc = bacc.Bacc()

    qT_d = nc.dram_tensor("qT", [DM, T], F16, kind="ExternalInput")
    kT_d = nc.dram_tensor("kT", [DM, S], F16, kind="ExternalInput")
    vaug_d = nc.dram_tensor("vaug", [S, H * (HD + 1)], F16, kind="ExternalInput")
    biasT_d = nc.dram_tensor("biasT", [H, S, T], F16, kind="ExternalInput")
    wT_d = nc.dram_tensor("wT", [DM, DM], F16, kind="ExternalInput")
    outb_d = nc.dram_tensor("outb", [P, ND], F32, kind="ExternalInput")
    outT_d = nc.dram_tensor("outT", [DM, T], F32, kind="ExternalOutput")

    with tile.TileContext(nc) as tc, nc.allow_low_precision(reason="fp16 matmul pipeline"):
        with (
            tc.tile_pool(name="weights", bufs=1) as wpool,
            tc.tile_pool(name="bias", bufs=3) as bpool,
            tc.tile_pool(name="rows", bufs=3) as rpool,
            tc.tile_pool(name="small", bufs=2) as spool,
            tc.tile_pool(name="osb", bufs=1) as opool_sb,
        ):
            # ---- persistent weight tiles (issue critical-path DMAs first) ----
            qT_t = [wpool.tile([P, T], F16, name=f"qT{c}", tag=f"qT{c}") for c in range(ND)]
            kT_t = [wpool.tile([P, S], F16, name=f"kT{c}", tag=f"kT{c}") for c in range(ND)]
            vaug_t = [wpool.tile([P, H * (HD + 1)], F16, name=f"va{c}", tag=f"va{c}") for c in range(NS)]
            wT_t = [wpool.tile([P, DM], F16, name=f"wT{c}", tag=f"wT{c}") for c in range(ND)]
            outb_t = wpool.tile([P, ND], F32, name="outb", tag="outb")
            eshift_t = wpool.tile([P, 1], F32, name="eshift", tag="eshift")
            nc.vector.memset(eshift_t[:], EXP_SHIFT)
            ones_t = wpool.tile([P, HD], F16, name="ones", tag="ones")
            nc.vector.memset(ones_t[:], 1.0)
            warm_t = wpool.tile([P, 512], F16, name="warm", tag="warm")
            nc.vector.memset(warm_t[:], 0.0)
            aflat_t = [wpool.tile([P, T], F16, name=f"af{c}", tag=f"af{c}") for c in range(ND)]

            nc.sync.dma_start(out=qT_t[0][:], in_=qT_d[0:P, :])
            nc.sync.dma_start(out=kT_t[0][:], in_=kT_d[0:P, :])
            nc.sync.dma_start(out=outb_t[:], in_=outb_d[:, :])

            with (
                tc.tile_pool(name="warmps", bufs=1, space="PSUM") as warmps,
                tc.tile_pool(name="scps", bufs=3, space="PSUM") as scps,
                tc.tile_pool(name="avps", bufs=3, space="PSUM") as avps,
                tc.tile_pool(name="bcps", bufs=1, space="PSUM") as bcps,
            ):
                wm_ps = warmps.tile([P, 512], F32, name="wm", tag="wm")
                for _ in range(24):
                    nc.tensor.matmul(wm_ps[:], warm_t[:, 0:P], warm_t[:],
                                     start=True, stop=True)
                for hp_i in range(H // 2):
                    h0 = 2 * hp_i
                    c2 = hp_i

                    bias_sb = bpool.tile([P, 2 * NS * T], F16, name="bias", tag="bias")
                    nc.sync.dma_start(
                        out=bias_sb[:].rearrange("p (h sc t) -> p h sc t", h=2, t=T),
                        in_=biasT_d[h0:h0 + 2].rearrange("h (sc p) t -> p h sc t", p=P),
                    )
                    if hp_i == 0:
                        for c in range(NS):
                            nc.sync.dma_start(out=vaug_t[c][:], in_=vaug_d[c * P:(c + 1) * P, :])
                    # stream later kT/qT chunks one pair ahead of first use
                    c_next = hp_i + 1
                    if c_next < ND:
                        nc.sync.dma_start(out=kT_t[c_next][:], in_=kT_d[c_next * P:(c_next + 1) * P, :])
                        nc.sync.dma_start(out=qT_t[c_next][:], in_=qT_d[c_next * P:(c_next + 1) * P, :])
                    # prefetch out_proj weights mid-stream
                    if hp_i == 5:
                        for c in range(ND):
                            nc.sync.dma_start(out=wT_t[c][:], in_=wT_d[c * P:(c + 1) * P, :])

                    expvs = [rpool.tile([P, NS * T], F16, name=f"expv{half}", tag=f"expv{half}")
                             for half in range(2)]
                    for sc2 in range(NS // 2):
                        pair_ps = []
                        for half in range(2):
                            hps = slice(half * HD, (half + 1) * HD)
                            sc_ps = scps.tile([P, 2 * T], F32, name="sc", tag="sc")
                            pair_ps.append(sc_ps)
                            for j in range(2):
                                sc = 2 * sc2 + j
                                nc.tensor.matmul(
                                    sc_ps[:, j * T:(j + 1) * T],
                                    kT_t[c2][hps, sc * P:(sc + 1) * P],
                                    qT_t[c2][hps, :],
                                    start=True, stop=True,
                                )
                        sl = slice(sc2 * 2 * T, (sc2 + 1) * 2 * T)
                        for half in range(2):
                            nc.scalar.activation(
                                expvs[half][:, sl], pair_ps[half][:], AF.Exp,
                                bias=eshift_t[:], scale=SCALE,
                            )
                            bsl = slice((half * NS + sc2 * 2) * T, (half * NS + sc2 * 2 + 2) * T)
                            nc.vector.tensor_mul(
                                expvs[half][:, sl], expvs[half][:, sl], bias_sb[:, bsl])

                    for half in range(2):
                        h = h0 + half
                        hp = slice(half * HD, (half + 1) * HD)
                        expv = expvs[half]
                        av_ps = avps.tile([HD + 1, T], F32, name="av", tag="av")
                        for sc in range(NS):
                            nc.tensor.matmul(
                                av_ps[:],
                                vaug_t[sc][:, h * (HD + 1):(h + 1) * (HD + 1)],
                                expv[:, sc * T:(sc + 1) * T],
                                start=(sc == 0), stop=(sc == NS - 1),
                            )
                        nc.tensor.matmul(wm_ps[:], warm_t[:, 0:P], warm_t[:],
                                         start=True, stop=True)
                        den_sb = spool.tile([1, T], F32, name="den_sb", tag="den_sb")
                        nc.vector.tensor_copy(den_sb[:], av_ps[HD:HD + 1, :])
                        rcp = spool.tile([1, T], F32, name="rcp", tag="rcp")
                        nc.vector.reciprocal_approx_fast(rcp[:], den_sb[:])
                        rcp16 = spool.tile([1, T], F16, name="rcp16", tag="rcp16")
                        nc.vector.tensor_copy(rcp16[:], rcp[:])
                        bc_ps = bcps.tile([HD, T], F32, name="bcp", tag="bcp")
                        nc.tensor.matmul(
                            bc_ps[:], ones_t[0:1, :], rcp16[:],
                            start=True, stop=True,
                        )
                        bc_sb = spool.tile([HD, T], F32, name="bc", tag="bc", bufs=4)
                        nc.scalar.copy(bc_sb[:], bc_ps[:])
                        nc.vector.tensor_mul(
                            aflat_t[c2][hp, :], av_ps[0:HD, :], bc_sb[:],
                        )

                # keep PE hot across the out_proj dependency boundary
                for _ in range(8):
                    nc.tensor.matmul(wm_ps[:], warm_t[:, 0:P], warm_t[:],
                                     start=True, stop=True)

            # ---- out_proj: outT[dout, t] = W^T @ attnflatT + out_b ----
            osb = opool_sb.tile([P, ND * T], F32, name="osb", tag="osb")
            with tc.tile_pool(name="ops", bufs=4, space="PSUM") as ops:
                for dc in range(ND):
                    o_ps = ops.tile([P, T], F32, name="o", tag="o")
                    for dinc in range(ND):
                        nc.tensor.matmul(
                            o_ps[:],
                            wT_t[dinc][:, dc * P:(dc + 1) * P],
                            aflat_t[dinc][:],
                            start=(dinc == 0), stop=(dinc == ND - 1),
                        )
                    nc.scalar.activation(
                        osb[:, dc * T:(dc + 1) * T], o_ps[:], AF.Identity,
                        bias=outb_t[:, dc:dc + 1],
                    )

            nc.sync.dma_start(
                out=outT_d.rearrange("(dc p) t -> p dc t", p=P),
                in_=osb[:].rearrange("p (dc t) -> p dc t", t=T),
            )

    nc.finalize()
    return nc


_NC = None


def _get_nc():
    global _NC
    if _NC is None:
        _NC = build_bass()
    return _NC


def _make_in_maps(query, key, value, attn_bias, key_padding_mask, out_w, out_b):
    query = np.asarray(query, dtype=np.float32)
    key = np.asarray(key, dtype=np.float32)
    value = np.asarray(value, dtype=np.float32)
    attn_bias = np.asarray(attn_bias, dtype=np.float32)
    mask = np.asarray(key_padding_mask).astype(bool)
    out_w = np.asarray(out_w, dtype=np.float32)
    out_b = np.asarray(out_b, dtype=np.float32)

    wT = np.ascontiguousarray(out_w.T).astype(NP16)
    outb = np.ascontiguousarray(out_b.reshape(ND, P).T)

    per_batch = {}
    for b in range(2):
        kT = np.ascontiguousarray(key[b].T).astype(NP16)
        vaug = np.ones((S, H * (HD + 1)), NP16)
        vaug.reshape(S, H, HD + 1)[:, :, :HD] = value[b].reshape(S, H, HD)
        per_batch[b] = (kT, vaug)

    in_maps = []
    for i in range(8):
        b, tc_i = divmod(i, 4)
        t0 = tc_i * T
        kT, vaug = per_batch[b]
        qT = np.ascontiguousarray(query[b, t0:t0 + T, :].T).astype(NP16)
        biasT = np.ascontiguousarray(
            attn_bias[b, :, t0:t0 + T, :].transpose(0, 2, 1)
        )
        biasT[:, mask[b], :] = -10000.0
        np.exp(biasT, out=biasT)
        in_maps.append({
            "qT": qT, "kT": kT, "vaug": vaug, "biasT": biasT.astype(NP16),
            "wT": wT, "outb": outb,
        })
    return in_maps


def run(inputs, trace=False, **run_kwargs):
    """Returns (output [2,1024,1024] f32, BassKernelResults)."""
    nc = _get_nc()
    in_maps = _make_in_maps(**inputs)
    res = run_bass_kernel_spmd(
        nc, in_maps, core_ids=list(range(8)), trace=trace, **run_kwargs
    )
    out = np.empty((2, S, DM), np.float32)
    for i, r in enumerate(res.results):
        b, tc_i = divmod(i, 4)
        out[b, tc_i * T:(tc_i + 1) * T, :] = r["outT"].T
    return out, res


def kernel(**inputs):
    out, _ = run(inputs, trace=False)
    return out


# revision 19
# speedup vs baseline: 1.0533x; 1.0533x over previous
"""Multi-head attention (no qkv proj) + out_proj, sharded over 8 TRN2 cores.

Sharding: data-parallel over (batch, T-chunk): core i handles batch i//4,
query rows [ (i%4)*256, (i%4+1)*256 ).  Each core computes its full output
rows (all 16 heads + out_proj) independently -- no collectives needed.

Layout strategy ("T on the free dim" everywhere, zero on-device transposes):
  scoresT[s, t]  = K_h @ Q_h^T        lhsT = kT[64d, 128s]   rhs = qT[64d, 256t]
  rows[s, t]     = scoresT*scale + biasT(+mask)   (mask folded into biasT on
                   host; no max-subtract needed: |scores+bias| <= ~10)
  exp[s, t]      = exp(rows - 2)  in fp16 (prescale keeps exp < 65504; the
                   constant cancels in the softmax normalization)
  av[d(+1), t]   = [V_h | 1]^T @ exp  (65th row = softmax denominator)
  attnflatT[din, t] = av[0:64] * bcast(1/denom)   (reciprocal batched over
                   4 heads; broadcast across partitions via GPSIMD)
  outT[dout, t]  = W^T-chunks^T @ attnflatT + out_b  (bias per-partition,
                   fused into the PSUM->SBUF copy on the scalar engine)

Host pre-transposes q/k/w/bias so every DMA is contiguous.  Matmul operands
are fp16 (full-rate PE path + fast weight load); accumulation is fp32 in
PSUM; softmax bias add runs in fp32.
"""

import ml_dtypes
import numpy as np

import concourse.bass as bass
import concourse.mybir as mybir
import concourse.tile as tile
from concourse import bacc
from concourse.bass_utils import run_bass_kernel_spmd

F32 = mybir.dt.float32
import os as _os
F16 = mybir.dt.bfloat16 if _os.environ.get("MM_BF16") else mybir.dt.float16
BIAS_F16 = True  # attn_bias streamed as f32 (precision) vs f16 (half DMA)
BIAS_DT = F16 if BIAS_F16 else F32
NP16 = ml_dtypes.bfloat16 if _os.environ.get("MM_BF16") else np.float16
BIAS_NP = NP16 if BIAS_F16 else np.float32

P = 128          # partitions
T = 256          # query rows per core
S = 1024         # key length
H = 16           # heads
HD = 64          # head dim
DM = 1024        # d_model
NS = S // P      # 8 s-chunks
ND = DM // P     # 8 d_model-chunks
SCALE = HD ** -0.5
EXP_SHIFT = -2.0  # exp(x-2): keeps exp outputs < 65504 for fp16

AF = mybir.ActivationFunctionType
ALU = mybir.AluOpType


def build_bass():
    nc = bacc.Bacc()

    qT_d = nc.dram_tensor("qT", [DM, T], F16, kind="ExternalInput")
    kT_d = nc.dram_tensor("kT", [DM, S], F16, kind="ExternalInput")
    vaug_d = nc.dram_tensor("vaug", [S, H * (HD + 1)], F16, kind="ExternalInput")
    biasT_d = nc.dram_tensor("biasT", [H, S, T], F16, kind="ExternalInput")
    wT_d = nc.dram_tensor("wT", [DM, DM], F16, kind="ExternalInput")
    outb_d = nc.dram_tensor("outb", [P, ND], F32, kind="ExternalInput")
    outT_d = nc.dram_tensor("outT", [DM, T], F32, kind="ExternalOutput")

    with tile.TileContext(nc) as tc, nc.allow_low_precision(reason="fp16 matmul pipeline"):
        with (
            tc.tile_pool(name="weights", bufs=1) as wpool,
            tc.tile_pool(name="bias", bufs=3) as bpool,
            tc.tile_pool(name="rows", bufs=3) as rpool,
            tc.tile_pool(name="small", bufs=2) as spool,
            tc.tile_pool(name="osb", bufs=1) as opool_sb,
        ):
            # ---- persistent weight tiles (issue critical-path DMAs first) ----
            qT_t = [wpool.tile([P, T], F16, name=f"qT{c}", tag=f"qT{c}") for c in range(ND)]
            kT_t = [wpool.tile([P, S], F16, name=f"kT{c}", tag=f"kT{c}") for c in range(ND)]
            vaug_t = [wpool.tile([P, H * (HD + 1)], F16, name=f"va{c}", tag=f"va{c}") for c in range(NS)]
            wT_t = [wpool.tile([P, DM], F16, name=f"wT{c}", tag=f"wT{c}") for c in range(ND)]
            outb_t = wpool.tile([P, ND], F32, name="outb", tag="outb")
            eshift_t = wpool.tile([P, 1], F32, name="eshift", tag="eshift")
            nc.vector.memset(eshift_t[:], EXP_SHIFT)
            ones_t = wpool.tile([P, HD], F16, name="ones", tag="ones")
            nc.vector.memset(ones_t[:], 1.0)
            warm_t = wpool.tile([P, 512], F16, name="warm", tag="warm")
            nc.vector.memset(warm_t[:], 0.0)
            aflat_t = [wpool.tile([P, T], F16, name=f"af{c}", tag=f"af{c}") for c in range(ND)]

            nc.sync.dma_start(out=qT_t[0][:], in_=qT_d[0:P, :])
            nc.sync.dma_start(out=kT_t[0][:], in_=kT_d[0:P, :])
            nc.sync.dma_start(out=outb_t[:], in_=outb_d[:, :])

            with (
                tc.tile_pool(name="warmps", bufs=1, space="PSUM") as warmps,
                tc.tile_pool(name="scps", bufs=3, space="PSUM") as scps,
                tc.tile_pool(name="avps", bufs=3, space="PSUM") as avps,
                tc.tile_pool(name="bcps", bufs=1, space="PSUM") as bcps,
            ):
                wm_ps = warmps.tile([P, 512], F32, name="wm", tag="wm")
                for _ in range(24):
                    nc.tensor.matmul(wm_ps[:], warm_t[:, 0:P], warm_t[:],
                                     start=True, stop=True)
                for h in range(H):
                    c2, half = divmod(h, 2)
                    hp = slice(half * HD, (half + 1) * HD)

                    bias_sb = bpool.tile([P, NS * T], F16, name="bias", tag="bias")
                    nc.sync.dma_start(
                        out=bias_sb[:].rearrange("p (sc t) -> p sc t", t=T),
                        in_=biasT_d[h].rearrange("(sc p) t -> p sc t", p=P),
                    )
                    if h == 0:
                        for c in range(NS):
                            nc.sync.dma_start(out=vaug_t[c][:], in_=vaug_d[c * P:(c + 1) * P, :])
                    # stream later kT/qT chunks two heads ahead of first use
                    c_next = h // 2 + 1
                    if h % 2 == 0 and c_next < ND:
                        nc.sync.dma_start(out=kT_t[c_next][:], in_=kT_d[c_next * P:(c_next + 1) * P, :])
                        nc.sync.dma_start(out=qT_t[c_next][:], in_=qT_d[c_next * P:(c_next + 1) * P, :])
                    # prefetch out_proj weights mid-stream
                    if h == 11:
                        for c in range(ND):
                            nc.sync.dma_start(out=wT_t[c][:], in_=wT_d[c * P:(c + 1) * P, :])

                    expv = rpool.tile([P, NS * T], F16, name="expv", tag="expv")
                    for sc2 in range(NS // 2):
                        sc_ps = scps.tile([P, 2 * T], F32, name="sc", tag="sc")
                        for j in range(2):
                            sc = 2 * sc2 + j
                            nc.tensor.matmul(
                                sc_ps[:, j * T:(j + 1) * T],
                                kT_t[c2][hp, sc * P:(sc + 1) * P],
                                qT_t[c2][hp, :],
                                start=True, stop=True,
                            )
                        sl = slice(sc2 * 2 * T, (sc2 + 1) * 2 * T)
                        nc.scalar.activation(
                            expv[:, sl], sc_ps[:], AF.Exp,
                            bias=eshift_t[:], scale=SCALE,
                        )
                        nc.vector.tensor_mul(expv[:, sl], expv[:, sl], bias_sb[:, sl])

                    av_ps = avps.tile([HD + 1, T], F32, name="av", tag="av")
                    for sc in range(NS):
                        nc.tensor.matmul(
                            av_ps[:],
                            vaug_t[sc][:, h * (HD + 1):(h + 1) * (HD + 1)],
                            expv[:, sc * T:(sc + 1) * T],
                            start=(sc == 0), stop=(sc == NS - 1),
                        )
                    nc.tensor.matmul(wm_ps[:], warm_t[:, 0:P], warm_t[:],
                                     start=True, stop=True)
                    den_sb = spool.tile([1, T], F32, name="den_sb", tag="den_sb")
                    nc.vector.tensor_copy(den_sb[:], av_ps[HD:HD + 1, :])
                    rcp = spool.tile([1, T], F32, name="rcp", tag="rcp")
                    nc.vector.reciprocal_approx_fast(rcp[:], den_sb[:])
                    rcp16 = spool.tile([1, T], F16, name="rcp16", tag="rcp16")
                    nc.vector.tensor_copy(rcp16[:], rcp[:])
                    bc_ps = bcps.tile([HD, T], F32, name="bcp", tag="bcp")
                    nc.tensor.matmul(
                        bc_ps[:], ones_t[0:1, :], rcp16[:],
                        start=True, stop=True,
                    )
                    bc_sb = spool.tile([HD, T], F32, name="bc", tag="bc", bufs=4)
                    nc.scalar.copy(bc_sb[:], bc_ps[:])
                    nc.vector.tensor_mul(
                        aflat_t[c2][hp, :], av_ps[0:HD, :], bc_sb[:],
                    )

                # keep PE hot across the out_proj dependency boundary
                for _ in range(8):
                    nc.tensor.matmul(wm_ps[:], warm_t[:, 0:P], warm_t[:],
                                     start=True, stop=True)

            # ---- out_proj: outT[dout, t] = W^T @ attnflatT + out_b ----
            osb = opool_sb.tile([P, ND * T], F32, name="osb", tag="osb")
            with tc.tile_pool(name="ops", bufs=4, space="PSUM") as ops:
                for dc in range(ND):
                    o_ps = ops.tile([P, T], F32, name="o", tag="o")
                    for dinc in range(ND):
                        nc.tensor.matmul(
                            o_ps[:],
                            wT_t[dinc][:, dc * P:(dc + 1) * P],
                            aflat_t[dinc][:],
                            start=(dinc == 0), stop=(dinc == ND - 1),
                        )
                    nc.scalar.activation(
                        osb[:, dc * T:(dc + 1) * T], o_ps[:], AF.Identity,
                        bias=outb_t[:, dc:dc + 1],
                    )

            nc.sync.dma_start(
                out=outT_d.rearrange("(dc p) t -> p dc t", p=P),
                in_=osb[:].rearrange("p (dc t) -> p dc t", t=T),
            )

    nc.finalize()
    return nc


_NC = None


def _get_nc():
    global _NC
    if _NC is None:
        _NC = build_bass()
    return _NC


def _make_in_maps(query, key, value, attn_bias, key_padding_mask, out_w, out_b):
    query = np.asarray(query, dtype=np.float32)
    key = np.asarray(key, dtype=np.float32)
    value = np.asarray(value, dtype=np.float32)
    attn_bias = np.asarray(attn_bias, dtype=np.float32)
    mask = np.asarray(key_padding_mask).astype(bool)
    out_w = np.asarray(out_w, dtype=np.float32)
    out_b = np.asarray(out_b, dtype=np.float32)

    wT = np.ascontiguousarray(out_w.T).astype(NP16)
    outb = np.ascontiguousarray(out_b.reshape(ND, P).T)

    per_batch = {}
    for b in range(2):
        kT = np.ascontiguousarray(key[b].T).astype(NP16)
        vaug = np.ones((S, H * (HD + 1)), NP16)
        vaug.reshape(S, H, HD + 1)[:, :, :HD] = value[b].reshape(S, H, HD)
        per_batch[b] = (kT, vaug)

    in_maps = []
    for i in range(8):
        b, tc_i = divmod(i, 4)
        t0 = tc_i * T
        kT, vaug = per_batch[b]
        qT = np.ascontiguousarray(query[b, t0:t0 + T, :].T).astype(NP16)
        biasT = np.ascontiguousarray(
            attn_bias[b, :, t0:t0 + T, :].transpose(0, 2, 1)
        )
        biasT[:, mask[b], :] = -10000.0
        np.exp(biasT, out=biasT)
        in_maps.append({
            "qT": qT, "kT": kT, "vaug": vaug, "biasT": biasT.astype(NP16),
            "wT": wT, "outb": outb,
        })
    return in_maps


def run(inputs, trace=False, **run_kwargs):
    """Returns (output [2,1024,1024] f32, BassKernelResults)."""
    nc = _get_nc()
    in_maps = _make_in_maps(**inputs)
    res = run_bass_kernel_spmd(
        nc, in_maps, core_ids=list(range(8)), trace=trace, **run_kwargs
    )
    out = np.empty((2, S, DM), np.float32)
    for i, r in enumerate(res.results):
        b, tc_i = divmod(i, 4)
        out[b, tc_i * T:(tc_i + 1) * T, :] = r["outT"].T
    return out, res


def kernel(**inputs):
    out, _ = run(inputs, trace=False)
    return out


# revision 20
# speedup vs baseline: 1.1365x; 1.0790x over previous
"""Multi-head attention (no qkv proj) + out_proj, sharded over 8 TRN2 cores.

Sharding: data-parallel over (batch, T-chunk): core i handles batch i//4,
query rows [ (i%4)*256, (i%4+1)*256 ).  Each core computes its full output
rows (all 16 heads + out_proj) independently -- no collectives needed.

Layout strategy ("T on the free dim" everywhere, zero on-device transposes):
  scoresT[s, t]  = K_h @ Q_h^T        lhsT = kT[64d, 128s]   rhs = qT[64d, 256t]
  rows[s, t]     = scoresT*scale + biasT(+mask)   (mask folded into biasT on
                   host; no max-subtract needed: |scores+bias| <= ~10)
  exp[s, t]      = exp(rows - 2)  in fp16 (prescale keeps exp < 65504; the
                   constant cancels in the softmax normalization)
  av[d(+1), t]   = [V_h | 1]^T @ exp  (65th row = softmax denominator)
  attnflatT[din, t] = av[0:64] * bcast(1/denom)   (reciprocal batched over
                   4 heads; broadcast across partitions via GPSIMD)
  outT[dout, t]  = W^T-chunks^T @ attnflatT + out_b  (bias per-partition,
                   fused into the PSUM->SBUF copy on the scalar engine)

Host pre-transposes q/k/w/bias so every DMA is contiguous.  Matmul operands
are fp16 (full-rate PE path + fast weight load); accumulation is fp32 in
PSUM; softmax bias add runs in fp32.
"""

import ml_dtypes
import numpy as np

import concourse.bass as bass
import concourse.mybir as mybir
import concourse.tile as tile
from concourse import bacc
from concourse.bass_utils import run_bass_kernel_spmd

F32 = mybir.dt.float32
import os as _os
F16 = mybir.dt.bfloat16 if _os.environ.get("MM_BF16") else mybir.dt.float16
BIAS_F16 = True  # attn_bias streamed as f32 (precision) vs f16 (half DMA)
BIAS_DT = F16 if BIAS_F16 else F32
NP16 = ml_dtypes.bfloat16 if _os.environ.get("MM_BF16") else np.float16
BIAS_NP = NP16 if BIAS_F16 else np.float32

P = 128          # partitions
T = 256          # query rows per core
S = 1024         # key length
H = 16           # heads
HD = 64          # head dim
DM = 1024        # d_model
NS = S // P      # 8 s-chunks
ND = DM // P     # 8 d_model-chunks
SCALE = HD ** -0.5
EXP_SHIFT = -2.0  # exp(x-2): keeps exp outputs < 65504 for fp16

AF = mybir.ActivationFunctionType
ALU = mybir.AluOpType


def build_bass():
    nc = bacc.Bacc()

    qT_d = nc.dram_tensor("qT", [DM, T], F16, kind="ExternalInput")
    kT_d = nc.dram_tensor("kT", [DM, S], F16, kind="ExternalInput")
    vaug_d = nc.dram_tensor("vaug", [S, H * (HD + 1)], F16, kind="ExternalInput")
    biasT_d = nc.dram_tensor("biasT", [H, S, T], F16, kind="ExternalInput")
    wT_d = nc.dram_tensor("wT", [DM, DM], F16, kind="ExternalInput")
    outb_d = nc.dram_tensor("outb", [P, ND], F32, kind="ExternalInput")
    outT_d = nc.dram_tensor("outT", [DM, T], F32, kind="ExternalOutput")

    with tile.TileContext(nc) as tc, nc.allow_low_precision(reason="fp16 matmul pipeline"):
        with (
            tc.tile_pool(name="weights", bufs=1) as wpool,
            tc.tile_pool(name="bias", bufs=3) as bpool,
            tc.tile_pool(name="rows", bufs=3) as rpool,
            tc.tile_pool(name="small", bufs=2) as spool,
            tc.tile_pool(name="osb", bufs=1) as opool_sb,
        ):
            # ---- persistent weight tiles (issue critical-path DMAs first) ----
            qT_t = [wpool.tile([P, T], F16, name=f"qT{c}", tag=f"qT{c}") for c in range(ND)]
            kT_t = [wpool.tile([P, S], F16, name=f"kT{c}", tag=f"kT{c}") for c in range(ND)]
            vaug_t = [wpool.tile([P, H * (HD + 1)], F16, name=f"va{c}", tag=f"va{c}") for c in range(NS)]
            wT_t = [wpool.tile([P, DM], F16, name=f"wT{c}", tag=f"wT{c}") for c in range(ND)]
            outb_t = wpool.tile([P, ND], F32, name="outb", tag="outb")
            eshift_t = wpool.tile([P, 1], F32, name="eshift", tag="eshift")
            nc.vector.memset(eshift_t[:], EXP_SHIFT)
            ones_t = wpool.tile([P, HD], F16, name="ones", tag="ones")
            nc.vector.memset(ones_t[:], 1.0)
            warm_t = wpool.tile([P, 512], F16, name="warm", tag="warm")
            nc.vector.memset(warm_t[:], 0.0)
            aflat_t = [wpool.tile([P, T], F16, name=f"af{c}", tag=f"af{c}") for c in range(ND)]

            nc.sync.dma_start(out=qT_t[0][:], in_=qT_d[0:P, :])
            nc.sync.dma_start(out=kT_t[0][:], in_=kT_d[0:P, :])
            nc.sync.dma_start(out=outb_t[:], in_=outb_d[:, :])

            with (
                tc.tile_pool(name="warmps", bufs=1, space="PSUM") as warmps,
                tc.tile_pool(name="scps", bufs=3, space="PSUM") as scps,
                tc.tile_pool(name="avps", bufs=3, space="PSUM") as avps,
                tc.tile_pool(name="bcps", bufs=1, space="PSUM") as bcps,
            ):
                wm_ps = warmps.tile([P, 512], F32, name="wm", tag="wm")
                for _ in range(24):
                    nc.tensor.matmul(wm_ps[:], warm_t[:, 0:P], warm_t[:],
                                     start=True, stop=True)
                for h in range(H):
                    c2, half = divmod(h, 2)
                    hp = slice(half * HD, (half + 1) * HD)

                    bias_sb = bpool.tile([P, NS * T], F16, name="bias", tag="bias")
                    nc.sync.dma_start(
                        out=bias_sb[:].rearrange("p (sc t) -> p sc t", t=T),
                        in_=biasT_d[h].rearrange("(sc p) t -> p sc t", p=P),
                    )
                    if h == 0:
                        for c in range(NS):
                            nc.sync.dma_start(out=vaug_t[c][:], in_=vaug_d[c * P:(c + 1) * P, :])
                    # stream later kT/qT chunks two heads ahead of first use
                    c_next = h // 2 + 1
                    if h % 2 == 0 and c_next < ND:
                        nc.sync.dma_start(out=kT_t[c_next][:], in_=kT_d[c_next * P:(c_next + 1) * P, :])
                        nc.sync.dma_start(out=qT_t[c_next][:], in_=qT_d[c_next * P:(c_next + 1) * P, :])
                    # prefetch out_proj weights mid-stream on the SWDGE
                    # queue so the HWDGE bias stream is not delayed
                    if h == 8:
                        for c in range(ND):
                            nc.gpsimd.dma_start(out=wT_t[c][:], in_=wT_d[c * P:(c + 1) * P, :])

                    expv = rpool.tile([P, NS * T], F16, name="expv", tag="expv")
                    for sc2 in range(NS // 2):
                        sc_ps = scps.tile([P, 2 * T], F32, name="sc", tag="sc")
                        for j in range(2):
                            sc = 2 * sc2 + j
                            nc.tensor.matmul(
                                sc_ps[:, j * T:(j + 1) * T],
                                kT_t[c2][hp, sc * P:(sc + 1) * P],
                                qT_t[c2][hp, :],
                                start=True, stop=True,
                            )
                        sl = slice(sc2 * 2 * T, (sc2 + 1) * 2 * T)
                        nc.scalar.activation(
                            expv[:, sl], sc_ps[:], AF.Exp,
                            bias=eshift_t[:], scale=SCALE,
                        )
                        nc.vector.tensor_mul(expv[:, sl], expv[:, sl], bias_sb[:, sl])

                    av_ps = avps.tile([HD + 1, T], F32, name="av", tag="av")
                    for sc in range(NS):
                        nc.tensor.matmul(
                            av_ps[:],
                            vaug_t[sc][:, h * (HD + 1):(h + 1) * (HD + 1)],
                            expv[:, sc * T:(sc + 1) * T],
                            start=(sc == 0), stop=(sc == NS - 1),
                        )
                    nc.tensor.matmul(wm_ps[:], warm_t[:, 0:P], warm_t[:],
                                     start=True, stop=True)
                    den_sb = spool.tile([1, T], F32, name="den_sb", tag="den_sb")
                    nc.vector.tensor_copy(den_sb[:], av_ps[HD:HD + 1, :])
                    rcp = spool.tile([1, T], F32, name="rcp", tag="rcp")
                    nc.vector.reciprocal_approx_fast(rcp[:], den_sb[:])
                    rcp16 = spool.tile([1, T], F16, name="rcp16", tag="rcp16")
                    nc.vector.tensor_copy(rcp16[:], rcp[:])
                    bc_ps = bcps.tile([HD, T], F32, name="bcp", tag="bcp")
                    nc.tensor.matmul(
                        bc_ps[:], ones_t[0:1, :], rcp16[:],
                        start=True, stop=True,
                    )
                    bc_sb = spool.tile([HD, T], F32, name="bc", tag="bc", bufs=4)
                    nc.scalar.copy(bc_sb[:], bc_ps[:])
                    nc.vector.tensor_mul(
                        aflat_t[c2][hp, :], av_ps[0:HD, :], bc_sb[:],
                    )

                # keep PE hot across the out_proj dependency boundary
                for _ in range(8):
                    nc.tensor.matmul(wm_ps[:], warm_t[:, 0:P], warm_t[:],
                                     start=True, stop=True)

            # ---- out_proj: outT[dout, t] = W^T @ attnflatT + out_b ----
            osb = opool_sb.tile([P, ND * T], F32, name="osb", tag="osb")
            with tc.tile_pool(name="ops", bufs=4, space="PSUM") as ops:
                for dc in range(ND):
                    o_ps = ops.tile([P, T], F32, name="o", tag="o")
                    for dinc in range(ND):
                        nc.tensor.matmul(
                            o_ps[:],
                            wT_t[dinc][:, dc * P:(dc + 1) * P],
                            aflat_t[dinc][:],
                            start=(dinc == 0), stop=(dinc == ND - 1),
                        )
                    nc.scalar.activation(
                        osb[:, dc * T:(dc + 1) * T], o_ps[:], AF.Identity,
                        bias=outb_t[:, dc:dc + 1],
                    )

            nc.sync.dma_start(
                out=outT_d.rearrange("(dc p) t -> p dc t", p=P),
                in_=osb[:].rearrange("p (dc t) -> p dc t", t=T),
            )

    nc.finalize()
    return nc


_NC = None


def _get_nc():
    global _NC
    if _NC is None:
        _NC = build_bass()
    return _NC


def _make_in_maps(query, key, value, attn_bias, key_padding_mask, out_w, out_b):
    query = np.asarray(query, dtype=np.float32)
    key = np.asarray(key, dtype=np.float32)
    value = np.asarray(value, dtype=np.float32)
    attn_bias = np.asarray(attn_bias, dtype=np.float32)
    mask = np.asarray(key_padding_mask).astype(bool)
    out_w = np.asarray(out_w, dtype=np.float32)
    out_b = np.asarray(out_b, dtype=np.float32)

    wT = np.ascontiguousarray(out_w.T).astype(NP16)
    outb = np.ascontiguousarray(out_b.reshape(ND, P).T)

    per_batch = {}
    for b in range(2):
        kT = np.ascontiguousarray(key[b].T).astype(NP16)
        vaug = np.ones((S, H * (HD + 1)), NP16)
        vaug.reshape(S, H, HD + 1)[:, :, :HD] = value[b].reshape(S, H, HD)
        per_batch[b] = (kT, vaug)

    in_maps = []
    for i in range(8):
        b, tc_i = divmod(i, 4)
        t0 = tc_i * T
        kT, vaug = per_batch[b]
        qT = np.ascontiguousarray(query[b, t0:t0 + T, :].T).astype(NP16)
        biasT = np.ascontiguousarray(
            attn_bias[b, :, t0:t0 + T, :].transpose(0, 2, 1)
        )
        biasT[:, mask[b], :] = -10000.0
        np.exp(biasT, out=biasT)
        in_maps.append({
            "qT": qT, "kT": kT, "vaug": vaug, "biasT": biasT.astype(NP16),
            "wT": wT, "outb": outb,
        })
    return in_maps


def run(inputs, trace=False, **run_kwargs):
    """Returns (output [2,1024,1024] f32, BassKernelResults)."""
    nc = _get_nc()
    in_maps = _make_in_maps(**inputs)
    res = run_bass_kernel_spmd(
        nc, in_maps, core_ids=list(range(8)), trace=trace, **run_kwargs
    )
    out = np.empty((2, S, DM), np.float32)
    for i, r in enumerate(res.results):
        b, tc_i = divmod(i, 4)
        out[b, tc_i * T:(tc_i + 1) * T, :] = r["outT"].T
    return out, res


def kernel(**inputs):
    out, _ = run(inputs, trace=False)
    return out


# revision 30
# speedup vs baseline: 1.4800x; 1.3023x over previous
"""Multi-head attention (no qkv proj) + out_proj, sharded over 8 TRN2 cores.

Sharding (per the head-parallel hint): core i handles batch b = i//4,
query rows tc = (i//2)%2 of 512, and head group hg = i%2 (8 of 16 heads).
out_proj weight is column-sharded over the head groups; the "all-reduce"
is a host-side partial-sum of the two head-group outputs at gather time.

Layout strategy ("T on the free dim" everywhere, zero on-device transposes):
  scoresT[s, t]  = K_h @ Q_h^T        lhsT = kT[64d, 128s]   rhs = qT[64d, 512t]
  expv[s, t]     = exp(scoresT*scale - 2) * exp(biasT)(+mask)  -- exp(bias)
                   precomputed on host in fp16 (mask rows exactly 0); the -2
                   prescale keeps fp16 exp in range and cancels in softmax
  av[d(+1), t]   = [V_h | 1]^T @ expv  (65th row = softmax denominator)
  attnflatT[din, t] = av[0:64] * bcast(1/denom)  (fast reciprocal on DVE;
                   broadcast across partitions via a K=1 PE matmul)
  outT[dout, t]  = W^T-chunks^T @ attnflatT + out_b  (bias on core hg=0 only)

Host pre-transposes q/k/w/bias so every DMA is contiguous.  Matmul operands
are fp16 (full-rate PE + fast weight load); PSUM accumulation is fp32.
Dummy matmuls on a scratch PSUM bank keep the PE HAM un-throttled (2.4 GHz)
through DMA-bound stretches.
"""

import numpy as np

import concourse.mybir as mybir
import concourse.tile as tile
from concourse import bacc
from concourse.bass_utils import run_bass_kernel_spmd

F32 = mybir.dt.float32
F16 = mybir.dt.float16
NP16 = np.float16

P = 128          # partitions
T = 512          # query rows per core
S = 1024         # key length
H = 8            # heads per core (of 16)
HD = 64          # head dim
DIN = H * HD     # local d_model slice (512)
NDIN = DIN // P  # 4 chunks
DM = 1024        # full d_model
NS = S // P      # 8 s-chunks
ND = DM // P     # 8 d_out chunks
SCALE = HD ** -0.5
EXP_SHIFT = -2.0  # exp(x-2): keeps fp16 exp outputs well inside range

AF = mybir.ActivationFunctionType
ALU = mybir.AluOpType


def build_bass():
    nc = bacc.Bacc()

    qT_d = nc.dram_tensor("qT", [DIN, T], F16, kind="ExternalInput")
    kT_d = nc.dram_tensor("kT", [DIN, S], F16, kind="ExternalInput")
    vaug_d = nc.dram_tensor("vaug", [S, H * (HD + 1)], F16, kind="ExternalInput")
    biasT_d = nc.dram_tensor("biasT", [H, S, T], F16, kind="ExternalInput")
    wT_d = nc.dram_tensor("wT", [DIN, DM], F16, kind="ExternalInput")
    outb_d = nc.dram_tensor("outb", [P, ND], F32, kind="ExternalInput")
    outT_d = nc.dram_tensor("outT", [DM, T], F32, kind="ExternalOutput")

    with tile.TileContext(nc) as tc, nc.allow_low_precision(reason="fp16 matmul pipeline"):
        with (
            tc.tile_pool(name="weights", bufs=1) as wpool,
            tc.tile_pool(name="bias", bufs=3) as bpool,
            tc.tile_pool(name="rows", bufs=3) as rpool,
            tc.tile_pool(name="small", bufs=2) as spool,
            tc.tile_pool(name="osb", bufs=1) as opool_sb,
        ):
            qT_t = [wpool.tile([P, T], F16, name=f"qT{c}", tag=f"qT{c}") for c in range(NDIN)]
            kT_t = [wpool.tile([P, S], F16, name=f"kT{c}", tag=f"kT{c}") for c in range(NDIN)]
            vaug_t = [wpool.tile([P, H * (HD + 1)], F16, name=f"va{c}", tag=f"va{c}") for c in range(NS)]
            wT_t = [wpool.tile([P, DM], F16, name=f"wT{c}", tag=f"wT{c}") for c in range(NDIN)]
            outb_t = wpool.tile([P, ND], F32, name="outb", tag="outb")
            eshift_t = wpool.tile([P, 1], F32, name="eshift", tag="eshift")
            nc.vector.memset(eshift_t[:], EXP_SHIFT)
            ones_t = wpool.tile([P, HD], F16, name="ones", tag="ones")
            nc.vector.memset(ones_t[:], 1.0)
            warm_t = wpool.tile([P, 512], F16, name="warm", tag="warm")
            nc.vector.memset(warm_t[:], 0.0)
            aflat_t = [wpool.tile([P, T], F16, name=f"af{c}", tag=f"af{c}") for c in range(NDIN)]

            nc.sync.dma_start(out=qT_t[0][:], in_=qT_d[0:P, :])
            nc.sync.dma_start(out=kT_t[0][:], in_=kT_d[0:P, :])
            nc.sync.dma_start(out=outb_t[:], in_=outb_d[:, :])

            with (
                tc.tile_pool(name="warmps", bufs=1, space="PSUM") as warmps,
                tc.tile_pool(name="scps", bufs=2, space="PSUM") as scps,
                tc.tile_pool(name="avps", bufs=2, space="PSUM") as avps,
                tc.tile_pool(name="bcps", bufs=1, space="PSUM") as bcps,
            ):
                wm_ps = warmps.tile([P, 512], F32, name="wm", tag="wm")
                for _ in range(24):
                    nc.tensor.matmul(wm_ps[:], warm_t[:, 0:P], warm_t[:],
                                     start=True, stop=True)

                for h in range(H):
                    c2, half = divmod(h, 2)
                    hp = slice(half * HD, (half + 1) * HD)
                    if h == 4:
                        # re-warm burst: recovers the HAM un-throttle if a
                        # DMA hiccup let the PE idle past the MID window
                        for _ in range(8):
                            nc.tensor.matmul(wm_ps[:], warm_t[:, 0:P], warm_t[:],
                                             start=True, stop=True)

                    bias_sb = bpool.tile([P, NS * T], F16, name="bias", tag="bias")
                    nc.sync.dma_start(
                        out=bias_sb[:].rearrange("p (sc t) -> p sc t", t=T),
                        in_=biasT_d[h].rearrange("(sc p) t -> p sc t", p=P),
                    )
                    if h == 0:
                        for c in range(NS):
                            nc.sync.dma_start(out=vaug_t[c][:], in_=vaug_d[c * P:(c + 1) * P, :])
                    c_next = h // 2 + 1
                    if h % 2 == 0 and c_next < NDIN:
                        nc.sync.dma_start(out=kT_t[c_next][:], in_=kT_d[c_next * P:(c_next + 1) * P, :])
                        nc.sync.dma_start(out=qT_t[c_next][:], in_=qT_d[c_next * P:(c_next + 1) * P, :])

                    expv = rpool.tile([P, NS * T], F16, name="expv", tag="expv")
                    for sc2 in range(NS // 2):
                        sc_ps = scps.tile([P, 2 * T], F32, name="sc", tag="sc")
                        for j in range(2):
                            sc = 2 * sc2 + j
                            nc.tensor.matmul(
                                sc_ps[:, j * T:(j + 1) * T],
                                kT_t[c2][hp, sc * P:(sc + 1) * P],
                                qT_t[c2][hp, :],
                                start=True, stop=True,
                            )
                        sl = slice(sc2 * 2 * T, (sc2 + 1) * 2 * T)
                        nc.scalar.activation(
                            expv[:, sl], sc_ps[:], AF.Exp,
                            bias=eshift_t[:], scale=SCALE,
                        )
                        nc.vector.tensor_mul(expv[:, sl], expv[:, sl], bias_sb[:, sl])

                    nc.tensor.matmul(wm_ps[:], warm_t[:, 0:P], warm_t[:],
                                     start=True, stop=True)
                    av_ps = avps.tile([HD + 1, T], F32, name="av", tag="av")
                    for sc in range(NS):
                        nc.tensor.matmul(
                            av_ps[:],
                            vaug_t[sc][:, h * (HD + 1):(h + 1) * (HD + 1)],
                            expv[:, sc * T:(sc + 1) * T],
                            start=(sc == 0), stop=(sc == NS - 1),
                        )
                    nc.tensor.matmul(wm_ps[:], warm_t[:, 0:P], warm_t[:],
                                     start=True, stop=True)
                    den_sb = spool.tile([1, T], F32, name="den_sb", tag="den_sb")
                    nc.vector.tensor_copy(den_sb[:], av_ps[HD:HD + 1, :])
                    rcp = spool.tile([1, T], F32, name="rcp", tag="rcp")
                    nc.vector.reciprocal_approx_fast(rcp[:], den_sb[:])
                    rcp16 = spool.tile([1, T], F16, name="rcp16", tag="rcp16")
                    nc.vector.tensor_copy(rcp16[:], rcp[:])
                    bc_ps = bcps.tile([HD, T], F32, name="bcp", tag="bcp")
                    nc.tensor.matmul(
                        bc_ps[:], ones_t[0:1, :], rcp16[:],
                        start=True, stop=True,
                    )
                    bc_sb = spool.tile([HD, T], F32, name="bc", tag="bc", bufs=4)
                    nc.scalar.copy(bc_sb[:], bc_ps[:])
                    nc.vector.tensor_mul(
                        aflat_t[c2][hp, :], av_ps[0:HD, :], bc_sb[:],
                    )

                # keep PE hot across the out_proj dependency boundary
                for _ in range(8):
                    nc.tensor.matmul(wm_ps[:], warm_t[:, 0:P], warm_t[:],
                                     start=True, stop=True)

            # ---- out_proj: outT[dout, t] = W^T-slice @ attnflatT (+ out_b) ----
            for c in range(NDIN):
                nc.sync.dma_start(out=wT_t[c][:], in_=wT_d[c * P:(c + 1) * P, :])

            osb = opool_sb.tile([P, ND * T], F32, name="osb", tag="osb")
            with tc.tile_pool(name="ops", bufs=4, space="PSUM") as ops:
                for dc in range(ND):
                    o_ps = ops.tile([P, T], F32, name="o", tag="o")
                    for dinc in range(NDIN):
                        nc.tensor.matmul(
                            o_ps[:],
                            wT_t[dinc][:, dc * P:(dc + 1) * P],
                            aflat_t[dinc][:],
                            start=(dinc == 0), stop=(dinc == NDIN - 1),
                        )
                    osl = slice(dc * T, (dc + 1) * T)
                    if dc % 2 == 0:
                        nc.scalar.activation(
                            osb[:, osl], o_ps[:], AF.Identity,
                            bias=outb_t[:, dc:dc + 1],
                        )
                    else:
                        nc.vector.tensor_scalar_add(
                            osb[:, osl], o_ps[:], outb_t[:, dc:dc + 1],
                        )
                    nc.sync.dma_start(
                        out=outT_d[dc * P:(dc + 1) * P, :],
                        in_=osb[:, osl],
                    )

    nc.finalize()
    return nc


_NC = None


def _get_nc():
    global _NC
    if _NC is None:
        _NC = build_bass()
    return _NC


def _core_index(b, tc_i, hg):
    return b * 4 + tc_i * 2 + hg


def _make_in_maps(query, key, value, attn_bias, key_padding_mask, out_w, out_b):
    query = np.asarray(query, dtype=np.float32)
    key = np.asarray(key, dtype=np.float32)
    value = np.asarray(value, dtype=np.float32)
    attn_bias = np.asarray(attn_bias, dtype=np.float32)
    mask = np.asarray(key_padding_mask).astype(bool)
    out_w = np.asarray(out_w, dtype=np.float32)
    out_b = np.asarray(out_b, dtype=np.float32)

    wT_full = np.ascontiguousarray(out_w.T).astype(NP16)   # [din, dout]
    outb = np.ascontiguousarray(out_b.reshape(ND, P).T)
    outb0 = np.zeros_like(outb)

    maps = [None] * 8
    for b in range(2):
        kT_full = np.ascontiguousarray(key[b].T).astype(NP16)  # [1024, 1024]
        for hg in range(2):
            hs = hg * H              # first global head of the group
            ds = hg * DIN            # first d_model row of the group
            vaug = np.ones((S, H * (HD + 1)), NP16)
            vaug.reshape(S, H, HD + 1)[:, :, :HD] = (
                value[b, :, ds:ds + DIN].reshape(S, H, HD))
            kT = np.ascontiguousarray(kT_full[ds:ds + DIN])
            wT = np.ascontiguousarray(wT_full[ds:ds + DIN])
            for tc_i in range(2):
                t0 = tc_i * T
                qT = np.ascontiguousarray(
                    query[b, t0:t0 + T, ds:ds + DIN].T).astype(NP16)
                biasT = np.ascontiguousarray(
                    attn_bias[b, hs:hs + H, t0:t0 + T, :].transpose(0, 2, 1))
                biasT[:, mask[b], :] = -10000.0
                np.exp(biasT, out=biasT)
                maps[_core_index(b, tc_i, hg)] = {
                    "qT": qT, "kT": kT, "vaug": vaug,
                    "biasT": biasT.astype(NP16),
                    "wT": wT, "outb": outb if hg == 0 else outb0,
                }
    return maps


def run(inputs, trace=False, **run_kwargs):
    """Returns (output [2,1024,1024] f32, BassKernelResults)."""
    nc = _get_nc()
    in_maps = _make_in_maps(**inputs)
    res = run_bass_kernel_spmd(
        nc, in_maps, core_ids=list(range(8)), trace=trace, **run_kwargs
    )
    out = np.empty((2, S, DM), np.float32)
    for b in range(2):
        for tc_i in range(2):
            part = (np.asarray(res.results[_core_index(b, tc_i, 0)]["outT"])
                    + np.asarray(res.results[_core_index(b, tc_i, 1)]["outT"]))
            out[b, tc_i * T:(tc_i + 1) * T, :] = part.T
    return out, res


def kernel(**inputs):
    out, _ = run(inputs, trace=False)
    return out
